# revision 1
# baseline (speedup 1.0000x reference)
"""Trainium2 Bass kernel for a 4-layer GRU stack with per-step additive
self-attention over the layer hiddens (FBRNN).

Strategy: data-parallel over batch B=64 across 8 NeuronCores (8 batch rows
per core, no cross-core communication inside the recurrence). Per core:

  - Everything lives in a [feature-on-partitions, batch-on-free] layout so
    the GRU elementwise runs on 128 DVE/ACT lanes.
  - GRU matmuls: stationary operand = bf16 weight tiles [128,128] (FWL),
    moving operand = bf16 activations [128, 8]. PSUM accumulates fp32.
  - Layer-0 input transform (x @ W_ih[0].T) has no recurrent dependency:
    it is precomputed for all T in a batched GEMM at kernel start (after an
    on-device embedding gather via indirect DMA + PE transposes), stored in
    DRAM, and streamed back 49KB/step.
  - sigmoid(x) = 0.5*tanh(0.5x)+0.5 so the whole kernel uses one ACT
    table set (exp_and_others: tanh+exp) -> no ~2.7us table switches.
  - T-loop: tc.For_i with 8 steps unrolled per iteration.
"""

import os
import numpy as np
import ml_dtypes

import concourse.bass as bass
import concourse.mybir as mybir
import concourse.tile as tile
from concourse import bacc
from concourse.bass import ds, ts
from concourse.bass_utils import run_bass_kernel_spmd
from concourse.masks import make_identity

F32 = mybir.dt.float32
BF16 = mybir.dt.bfloat16
I32 = mybir.dt.int32
AF = mybir.ActivationFunctionType
ALU = mybir.AluOpType

T, B = 512, 64
V, E, H, L, A = 32000, 512, 512, 4, 256
NCORES = 8
BC = B // NCORES            # 8 batch rows per core
TOK = T * BC                # 4096 tokens per core, (t, b) order
G3 = 3 * H                  # 1536 gate rows
MCH = G3 // 128             # 12 gate chunks
KCH = E // 128              # 4 contraction chunks (E == H)
ACH = A // 128              # 2 attention chunks
HT = H // 128               # 4 hidden chunks
UNROLL = 8
SLAB = 512                  # tokens per prologue gemm slab
DEBUG_H = False             # add per-step dump of the full h state

# attention pair-block offsets for i=0..2 (i=3 is identity); block i holds
# columns (b, k) for k in [i, 4), b-major; block size (4-i)*BC
_OFF = [0, 4 * BC, 7 * BC]
_ETOT = 9 * BC              # 72


def _bcast(ap, dim, count):
    """Insert a [step=0, count] free dim at position `dim` (0=partition)."""
    new = list(ap.ap)
    new.insert(dim, [0, count])
    return bass.AP(tensor=ap.tensor, offset=ap.offset, ap=new)


def _build_kernel():
    nc = bacc.Bacc("TRN2", target_bir_lowering=False, debug=False)

    tokens_d = nc.dram_tensor("tokens32", [TOK // 128, 128], I32, kind="ExternalInput").ap()
    emb_d = nc.dram_tensor("embbf", [V, E], BF16, kind="ExternalInput").ap()
    wih0_d = nc.dram_tensor("wih0", [128, KCH, MCH, 128], BF16, kind="ExternalInput").ap()
    wih_d = nc.dram_tensor("wih", [L - 1, 128, KCH, MCH, 128], BF16, kind="ExternalInput").ap()
    whh_d = nc.dram_tensor("whh", [L, 128, KCH, MCH, 128], BF16, kind="ExternalInput").ap()
    wa_d = nc.dram_tensor("wa", [L, 128, KCH, ACH, 128], BF16, kind="ExternalInput").ap()
    va_d = nc.dram_tensor("vastk", [128, ACH, L], BF16, kind="ExternalInput").ap()
    ba_d = nc.dram_tensor("bastk", [128, ACH, L], F32, kind="ExternalInput").ap()
    brz_d = nc.dram_tensor("brz", [L, 128, 8], F32, kind="ExternalInput").ap()
    bin_d = nc.dram_tensor("bin", [L, 128, HT], F32, kind="ExternalInput").ap()
    bhn_d = nc.dram_tensor("bhn", [L, 128, HT], F32, kind="ExternalInput").ap()
    out_d = nc.dram_tensor("out", [T * BC, H], F32, kind="ExternalOutput").ap()
    global _dbg_d
    _dbg_d = None
    if DEBUG_H:
        _dbg_d = nc.dram_tensor("dbg", [T, 2, 128, HT * BC * L], F32,
                                kind="ExternalOutput").ap()

    with tile.TileContext(nc) as tc:
        _emit(tc, nc, tokens_d, emb_d, wih0_d, wih_d, whh_d, wa_d, va_d, ba_d,
              brz_d, bin_d, bhn_d, out_d)
    nc.compile()
    return nc


def _emit(tc, nc, tokens_d, emb_d, wih0_d, wih_d, whh_d, wa_d, va_d, ba_d,
          brz_d, bin_d, bhn_d, out_d):
    from contextlib import ExitStack

    ctx = ExitStack()
    with ctx:
        wpool = ctx.enter_context(tc.tile_pool(name="weights", bufs=1))
        state = ctx.enter_context(tc.tile_pool(name="state", bufs=1))
        dram = ctx.enter_context(tc.tile_pool(name="dram", bufs=1, space="DRAM"))

        # ---- resident weights -------------------------------------------
        wih0_sb = wpool.tile([128, KCH, MCH, 128], BF16, tag="wih0")
        nc.sync.dma_start(out=wih0_sb, in_=wih0_d)
        wih_sb = []
        for l in range(L - 1):
            w = wpool.tile([128, KCH, MCH, 128], BF16, tag=f"wih{l}")
            nc.sync.dma_start(out=w, in_=wih_d[l])
            wih_sb.append(w)
        whh_sb = []
        for l in range(L):
            w = wpool.tile([128, KCH, MCH, 128], BF16, tag=f"whh{l}")
            nc.sync.dma_start(out=w, in_=whh_d[l])
            whh_sb.append(w)
        wa_sb = []
        for i in range(L):
            w = wpool.tile([128, KCH, ACH, 128], BF16, tag=f"wa{i}")
            nc.sync.dma_start(out=w, in_=wa_d[i])
            wa_sb.append(w)
        va_sb = wpool.tile([128, ACH, L], BF16, tag="va")
        nc.sync.dma_start(out=va_sb, in_=va_d)
        ba_sb = wpool.tile([128, ACH, L], F32, tag="ba")
        nc.sync.dma_start(out=ba_sb, in_=ba_d)
        brz_sb = wpool.tile([128, L, 8], F32, tag="brz")
        nc.sync.dma_start(out=brz_sb, in_=brz_d.rearrange("l p m -> p l m"))
        bin_sb = wpool.tile([128, L, HT], F32, tag="bin")
        nc.sync.dma_start(out=bin_sb, in_=bin_d.rearrange("l p m -> p l m"))
        bhn_sb = wpool.tile([128, L, HT], F32, tag="bhn")
        nc.sync.dma_start(out=bhn_sb, in_=bhn_d.rearrange("l p m -> p l m"))

        ident = wpool.tile([128, 128], BF16, tag="ident")
        make_identity(nc, ident)
        ones_sb = wpool.tile([1, 128], BF16, tag="ones")
        nc.vector.memset(ones_sb, 1.0)

        # ---- recurrent state --------------------------------------------
        # layout: [128 part, HT, BC, L]
        h_f32 = state.tile([128, HT, BC, L], F32, tag="h_f32")
        h_bf = state.tile([128, HT, BC, L], BF16, tag="h_bf")
        new_f32 = state.tile([128, HT, BC, L], F32, tag="new_f32")
        new_bf = state.tile([128, HT, BC, L], BF16, tag="new_bf")
        nc.vector.memset(h_f32, 0.0)
        nc.vector.memset(h_bf, 0.0)

        # gi0[m, p, tok] fp32: precomputed x @ W_ih[0].T (no bias)
        gi0_dram = dram.tile([MCH, 128, TOK], F32, tag="gi0")

        # ---- prologue: embedding gather + layer-0 input GEMM ------------
        with tc.tile_pool(name="prol", bufs=2) as prol, \
             tc.tile_pool(name="prol_ps", bufs=2, space="PSUM") as prol_ps, \
             tc.tile_pool(name="gemm_ps", bufs=2, space="PSUM") as gemm_ps, \
             tc.tile_pool(name="evac", bufs=2) as evac, \
             tc.tile_pool(name="x0t", bufs=2) as x0tp:
            for slab in range(TOK // SLAB):
                x0t = x0tp.tile([128, KCH, SLAB], BF16, tag="x0t")
                for g in range(SLAB // 128):
                    gt = slab * (SLAB // 128) + g
                    tok_sb = prol.tile([128, 1], I32, tag="tok")
                    nc.sync.dma_start(out=tok_sb, in_=tokens_d[gt, :, None])
                    x0 = prol.tile([128, E], BF16, tag="x0")
                    nc.gpsimd.indirect_dma_start(
                        out=x0, out_offset=None, in_=emb_d,
                        in_offset=bass.IndirectOffsetOnAxis(ap=tok_sb[:, 0:1], axis=0),
                    )
                    for k in range(KCH):
                        pst = prol_ps.tile([128, 128], BF16, space="PSUM", tag="pst")
                        nc.tensor.transpose(out=pst, in_=x0[:, ts(k, 128)], identity=ident)
                        nc.vector.tensor_copy(out=x0t[:, k, ts(g, 128)], in_=pst)
                for m in range(MCH):
                    ps = gemm_ps.tile([128, SLAB], F32, space="PSUM", tag="g0ps")
                    for k in range(KCH):
                        nc.tensor.matmul(
                            out=ps, lhsT=wih0_sb[:, k, m, :], rhs=x0t[:, k, :],
                            start=(k == 0), stop=(k == KCH - 1),
                        )
                    ev = evac.tile([128, SLAB], F32, tag="ev")
                    nc.scalar.activation(out=ev, in_=ps, func=AF.Copy)
                    nc.sync.dma_start(out=gi0_dram[m, :, ts(slab, SLAB)], in_=ev)

        # ---- main recurrence --------------------------------------------
        loop_pools = ExitStack()
        with loop_pools:
            gip = loop_pools.enter_context(tc.tile_pool(name="gi", bufs=3))
            pgp = loop_pools.enter_context(tc.tile_pool(name="pg", bufs=4, space="PSUM"))
            ep = loop_pools.enter_context(tc.tile_pool(name="elem", bufs=3))
            up = loop_pools.enter_context(tc.tile_pool(name="ups", bufs=1, space="PSUM"))
            ep2 = loop_pools.enter_context(tc.tile_pool(name="eps", bufs=1, space="PSUM"))
            ep3 = loop_pools.enter_context(tc.tile_pool(name="abcps", bufs=1, space="PSUM"))
            ap_ = loop_pools.enter_context(tc.tile_pool(name="attn", bufs=2))
            pp = loop_pools.enter_context(tc.tile_pool(name="prod", bufs=2))

            with tc.For_i(0, TOK, BC * UNROLL,
                          hint_engines=(mybir.EngineType.PE,
                                        mybir.EngineType.DVE,
                                        mybir.EngineType.Activation)) as iv:
                for u in range(UNROLL):
                    _step(tc, nc, iv, u, gip, pgp, ep, up, ep2, ep3, ap_, pp,
                          wih_sb, whh_sb, wa_sb, va_sb, ba_sb, brz_sb, bin_sb,
                          bhn_sb, ones_sb, h_f32, h_bf, new_f32, new_bf,
                          gi0_dram, out_d)


def _step(tc, nc, iv, u, gip, pgp, ep, up, ep2, ep3, ap_, pp,
          wih_sb, whh_sb, wa_sb, va_sb, ba_sb, brz_sb, bin_sb, bhn_sb,
          ones_sb, h_f32, h_bf, new_f32, new_bf, gi0_dram, out_d):
    tb0 = iv + u * BC  # token index of (t, b=0)

    # stream in the precomputed layer-0 gi for this step: [128, MCH, BC]
    gi_sb = gip.tile([128, MCH, BC], F32, tag="gi0s")
    nc.sync.dma_start(
        out=gi_sb,
        in_=gi0_dram[:, :, ds(tb0, BC)].rearrange("m p b -> p m b"),
    )

    # psum gate tiles per layer: slots 0..7 = gh r,z; 8..11 = gh n-part;
    # slots 12..19 = gi r,z; 20..23 = gi n-part.  [128, 24, BC] = 1 bank.
    pg = [pgp.tile([128, 24, BC], F32, space="PSUM", tag="pg", name=f"pg{_l}")
          for _l in range(L)]

    def mm_gh(l):
        for m in range(MCH):
            for k in range(KCH):
                nc.tensor.matmul(
                    out=pg[l][:, m, :],
                    lhsT=whh_sb[l][:, k, m, :],
                    rhs=h_bf[:, k, :, l],
                    start=(k == 0) and m == 0,
                    stop=(k == KCH - 1) and m == MCH - 1,
                    skip_group_check=True,
                )

    def mm_gi(l):  # l >= 1; input = new[l-1]
        for m in range(MCH):
            for k in range(KCH):
                nc.tensor.matmul(
                    out=pg[l][:, 12 + m, :],
                    lhsT=wih_sb[l - 1][:, k, m, :],
                    rhs=new_bf[:, k, :, l - 1],
                    start=(k == 0) and m == 0,
                    stop=(k == KCH - 1) and m == MCH - 1,
                    skip_group_check=True,
                )

    def elem(l):
        # rz = sigmoid(gi_rz + gh_rz + b_rz) via 0.5*tanh(0.5x)+0.5
        # (walrus: each TensorTensor may read at most one PSUM operand)
        girz = gi_sb[:, 0:8, :] if l == 0 else pg[l][:, 12:20, :]
        rzb = ep.tile([128, 8, BC], F32, tag="rzb")
        nc.vector.tensor_tensor(out=rzb, in0=pg[l][:, 0:8, :],
                                in1=_bcast(brz_sb[:, l, :], 2, BC), op=ALU.add)
        nc.vector.tensor_tensor(out=rzb, in0=rzb, in1=girz, op=ALU.add)
        trz = ep.tile([128, 8, BC], F32, tag="trz")
        nc.scalar.activation(out=trz, in_=rzb, func=AF.Tanh, scale=0.5)
        rz = ep.tile([128, 8, BC], F32, tag="rz")
        nc.vector.tensor_scalar(out=rz, in0=trz, scalar1=0.5, scalar2=0.5,
                                op0=ALU.mult, op1=ALU.add)
        # n = tanh(gi_n + b_in + r * (gh_n + b_hn))
        hnb = ep.tile([128, HT, BC], F32, tag="hnb")
        nc.vector.tensor_tensor(out=hnb, in0=pg[l][:, 8:12, :],
                                in1=_bcast(bhn_sb[:, l, :], 2, BC), op=ALU.add)
        rh = ep.tile([128, HT, BC], F32, tag="rh")
        nc.vector.tensor_tensor(out=rh, in0=rz[:, 0:4, :], in1=hnb, op=ALU.mult)
        np1 = ep.tile([128, HT, BC], F32, tag="np1")
        gin = gi_sb[:, 8:12, :] if l == 0 else pg[l][:, 20:24, :]
        nc.vector.tensor_tensor(out=np1, in0=rh, in1=gin, op=ALU.add)
        np2 = ep.tile([128, HT, BC], F32, tag="np2")
        nc.vector.tensor_tensor(out=np2, in0=np1,
                                in1=_bcast(bin_sb[:, l, :], 2, BC), op=ALU.add)
        n = ep.tile([128, HT, BC], F32, tag="n")
        nc.scalar.activation(out=n, in_=np2, func=AF.Tanh)
        # new = n + z*(h - n)
        d = ep.tile([128, HT, BC], F32, tag="d")
        nc.vector.tensor_tensor(out=d, in0=h_f32[:, :, :, l], in1=n, op=ALU.subtract)
        zd = ep.tile([128, HT, BC], F32, tag="zd")
        nc.vector.tensor_tensor(out=zd, in0=rz[:, 4:8, :], in1=d, op=ALU.mult)
        nc.vector.tensor_tensor(out=new_f32[:, :, :, l], in0=n, in1=zd, op=ALU.add)
        nc.vector.tensor_copy(out=new_bf[:, :, :, l], in_=new_f32[:, :, :, l])

    # PE order: gh0, gh1, gi1, gh2, gi2, gh3, gi3 (gi[l] gated on elem[l-1])
    mm_gh(0)
    mm_gh(1)
    elem(0)
    mm_gi(1)
    mm_gh(2)
    elem(1)
    mm_gi(2)
    mm_gh(3)
    elem(2)
    mm_gi(3)
    elem(3)

    # ---- attention combine ------------------------------------------
    # u[i,k] = Wa[i].T @ new[k]  for i<3, k>=i; columns (b, k) per block i
    u_ps = up.tile([128, ACH, _ETOT], F32, space="PSUM", tag="ups")
    for i in range(3):
        sz = (L - i) * BC
        for a2 in range(ACH):
            for k in range(KCH):
                nc.tensor.matmul(
                    out=u_ps[:, a2, _OFF[i]:_OFF[i] + sz],
                    lhsT=wa_sb[i][:, k, a2, :],
                    rhs=new_bf[:, k, :, i:L],
                    start=(k == 0), stop=(k == KCH - 1),
                    skip_group_check=True,
                )
    u_sb = ap_.tile([128, ACH, _ETOT], F32, tag="usb")
    for i in range(3):
        sz = (L - i) * BC
        nc.vector.tensor_tensor(
            out=u_sb[:, :, _OFF[i]:_OFF[i] + sz],
            in0=u_ps[:, :, _OFF[i]:_OFF[i] + sz],
            in1=_bcast(ba_sb[:, :, i], 2, sz),
            op=ALU.add,
        )
    ut = ap_.tile([128, ACH, _ETOT], BF16, tag="ut")
    nc.scalar.activation(out=ut, in_=u_sb, func=AF.Tanh)
    e_ps = ep2.tile([1, _ETOT], F32, space="PSUM", tag="eps")
    for i in range(3):
        sz = (L - i) * BC
        for a2 in range(ACH):
            nc.tensor.matmul(out=e_ps[0:1, _OFF[i]:_OFF[i] + sz],
                             lhsT=va_sb[:, a2, i:i + 1],
                             rhs=ut[:, a2, _OFF[i]:_OFF[i] + sz],
                             start=(a2 == 0), stop=(a2 == ACH - 1),
                             skip_group_check=True)
    ee = ap_.tile([1, _ETOT], F32, tag="ee")
    nc.scalar.activation(out=ee, in_=e_ps, func=AF.Exp)
    s_all = ap_.tile([1, 4, BC], F32, tag="sall")
    for i in range(3):
        kk = L - i
        nc.vector.tensor_reduce(
            out=s_all[0:1, i, :],
            in_=ee[0:1, _OFF[i]:_OFF[i] + kk * BC].rearrange(
                "p (b k) -> p b k", k=kk),
            axis=mybir.AxisListType.X, op=ALU.add,
        )
    rs = ap_.tile([1, 4, BC], F32, tag="rs")
    nc.vector.reciprocal(out=rs[0:1, 0:3, :], in_=s_all[0:1, 0:3, :])
    a_bf = ap_.tile([1, _ETOT], BF16, tag="abf")
    for i in range(3):
        kk = L - i
        nc.vector.tensor_tensor(
            out=a_bf[0:1, _OFF[i]:_OFF[i] + kk * BC].rearrange(
                "p (b k) -> p b k", k=kk),
            in0=ee[0:1, _OFF[i]:_OFF[i] + kk * BC].rearrange(
                "p (b k) -> p b k", k=kk),
            in1=_bcast(rs[0:1, i, :], 2, kk),
            op=ALU.mult,
        )
    abc_ps = ep3.tile([128, _ETOT], F32, space="PSUM", tag="abc")
    nc.tensor.matmul(out=abc_ps, lhsT=ones_sb, rhs=a_bf, start=True, stop=True)
    for i in range(3):
        kk = L - i
        prod = pp.tile([128, HT, BC, L], F32, tag="prod")
        av = abc_ps[:, _OFF[i]:_OFF[i] + kk * BC].rearrange("p (b k) -> p b k", k=kk)
        nc.vector.tensor_tensor(
            out=prod[:, :, :, 0:kk],
            in0=new_f32[:, :, :, i:L],
            in1=_bcast(av, 1, HT),
            op=ALU.mult,
        )
        nc.vector.tensor_reduce(out=h_f32[:, :, :, i], in_=prod[:, :, :, 0:kk],
                                axis=mybir.AxisListType.X, op=ALU.add)
        nc.vector.tensor_copy(out=h_bf[:, :, :, i], in_=h_f32[:, :, :, i])
    # i = 3: softmax over a single element -> h_next[3] = new[3]
    nc.vector.tensor_copy(out=h_f32[:, :, :, 3], in_=new_f32[:, :, :, 3])
    nc.vector.tensor_copy(out=h_bf[:, :, :, 3], in_=new_f32[:, :, :, 3])

    # output row block: out[(t,b), :] for this step's 8 batch rows
    if DEBUG_H:
        # row offset t*256 = iv*32 + u*256 (iv counts tokens, 8/step)
        nc.sync.dma_start(
            out=_dbg_d.rearrange("t s p f -> (t s p) f")[
                ds(iv * 32 + u * 256, 128), :],
            in_=h_f32.rearrange("p ht b l -> p (ht b l)"),
        )
        nc.sync.dma_start(
            out=_dbg_d.rearrange("t s p f -> (t s p) f")[
                ds(iv * 32 + u * 256 + 128, 128), :],
            in_=new_f32.rearrange("p ht b l -> p (ht b l)"),
        )
    out_stage = ap_.tile([128, BC, HT], F32, tag="ostage")
    nc.vector.tensor_copy(out=out_stage,
                          in_=h_f32[:, :, :, 3].rearrange("p ht b -> p b ht"))
    nc.sync.dma_start(
        out=out_d[ds(tb0, BC), :].rearrange("b (ht p) -> p b ht", p=128),
        in_=out_stage,
    )


_NC_CACHE = {}


def _get_nc():
    if "nc" not in _NC_CACHE:
        _NC_CACHE["nc"] = _build_kernel()
    return _NC_CACHE["nc"]


def _prep_inputs(tokens, emb, W_ih, W_hh, b_ih, b_hh, Wa, ba, va):
    """Host-side input marshalling (weight layout/dtype only, no compute)."""
    bf = ml_dtypes.bfloat16
    emb_bf = np.ascontiguousarray(np.asarray(emb, np.float32).astype(bf))

    def lhsT_layout(wT):  # [K, M] -> [128, KCH, MCH, 128]
        K, M = wT.shape
        return np.ascontiguousarray(
            wT.reshape(K // 128, 128, M // 128, 128).transpose(1, 0, 2, 3).astype(bf))

    wih_t = [lhsT_layout(np.asarray(W_ih[l], np.float32).T) for l in range(L)]
    whh_t = [lhsT_layout(np.asarray(W_hh[l], np.float32).T) for l in range(L)]
    wa_t = [lhsT_layout(np.asarray(Wa[i], np.float32)) for i in range(L)]
    va_s = np.ascontiguousarray(
        np.asarray(va, np.float32).T.reshape(ACH, 128, L).transpose(1, 0, 2).astype(bf))

    bsum = np.asarray(b_ih, np.float32) + np.asarray(b_hh, np.float32)
    brz = np.ascontiguousarray(
        bsum[:, :1024].reshape(L, 8, 128).transpose(0, 2, 1))
    bin_ = np.ascontiguousarray(
        np.asarray(b_ih, np.float32)[:, 1024:].reshape(L, HT, 128).transpose(0, 2, 1))
    bhn = np.ascontiguousarray(
        np.asarray(b_hh, np.float32)[:, 1024:].reshape(L, HT, 128).transpose(0, 2, 1))

    ba_s = np.ascontiguousarray(
        np.asarray(ba, np.float32).T.reshape(ACH, 128, L).transpose(1, 0, 2))
    return emb_bf, wih_t, whh_t, wa_t, va_s, ba_s, brz, bin_, bhn


def kernel(tokens, emb, W_ih, W_hh, b_ih, b_hh, Wa, ba, va):
    nc = _get_nc()
    emb_bf, wih_t, whh_t, wa_t, va_s, ba_s, brz, bin_, bhn = _prep_inputs(
        tokens, emb, W_ih, W_hh, b_ih, b_hh, Wa, ba, va)

    tok = np.asarray(tokens).astype(np.int32)  # [T, B]
    wih_arr = np.stack(wih_t[1:])
    whh_arr = np.stack(whh_t)
    wa_arr = np.stack(wa_t)

    in_maps = []
    for c in range(NCORES):
        tok_c = np.ascontiguousarray(
            tok[:, c * BC:(c + 1) * BC]).reshape(TOK // 128, 128)
        in_maps.append({
            "tokens32": tok_c,
            "embbf": emb_bf,
            "wih0": wih_t[0],
            "wih": wih_arr,
            "whh": whh_arr,
            "wa": wa_arr,
            "vastk": va_s,
            "bastk": ba_s,
            "brz": brz,
            "bin": bin_,
            "bhn": bhn,
        })

    trace = bool(int(os.environ.get("KERNEL_TRACE", "0")))
    res = run_bass_kernel_spmd(nc, in_maps, core_ids=list(range(NCORES)),
                               trace=trace)
    if trace:
        _NC_CACHE["last_exec_time_ns"] = res.exec_time_ns
        _NC_CACHE["last_results"] = res

    outs = []
    for c in range(NCORES):
        o = res.results[c]["out"].reshape(T, BC, H)
        outs.append(o)
    return np.concatenate(outs, axis=1)



# revision 11
# speedup vs baseline: 470.9569x; 470.9569x over previous
"""Trainium2 Bass kernel for a 4-layer GRU stack with per-step additive
self-attention over the layer hiddens (FBRNN).

Strategy: data-parallel over batch B=64 across 8 NeuronCores (8 batch rows
per core, no cross-core communication inside the recurrence). Per core:

  - [feature-on-partitions, batch-on-free] layout so the GRU elementwise
    runs on 128 DVE/ACT lanes.
  - GRU matmuls: stationary = bf16 weight tiles [128,128] (FWL), moving =
    bf16 activations [128, 8]. PSUM accumulates fp32.
  - Gate biases are injected into PSUM by a tiny leading matmul
    (bias_lhsT.T @ mask), so the elementwise path reads bias-included
    gate pre-activations straight from PSUM.
  - sigmoid(x) = (1 + tanh(x/2))/2 is folded into scalar_tensor_tensor
    ops: the state is stored as h* = h/2 (W_hh r/z rows pre-scaled by 2,
    b_hn by 0.5 on the host), which makes every gate product a single
    fused (t +- 1) * y DVE op.  Only tanh/exp ACT tables are used.
  - Layer-0 input transform (x @ W_ih[0].T) has no recurrent dependency:
    precomputed for all T in a batched GEMM at kernel start (on-device
    embedding gather via indirect DMA + PE transposes), stored in DRAM
    bf16, streamed back 24.5KB/step and accumulated into PSUM via an
    identity matmul.
  - Attention (i<3) runs once per step; the i=3 combine is h3 = new3.
    The attention for step t is *emitted* at the top of step t+1 so its
    PE work interleaves with the next step's early (h-independent) gh[3]
    matmuls; the very last step's attention is never emitted (the output
    only needs new[3]).
  - T-loop: tc.For_i with UNROLL steps per iteration.
"""

import os
import numpy as np
import ml_dtypes

import concourse.bass as bass
import concourse.mybir as mybir
import concourse.tile as tile
from concourse import bacc
from concourse.bass import ds, ts
from concourse.bass_utils import run_bass_kernel_spmd
from concourse.masks import make_identity

F32 = mybir.dt.float32
BF16 = mybir.dt.bfloat16
I32 = mybir.dt.int32
AF = mybir.ActivationFunctionType
ALU = mybir.AluOpType

T, B = 512, 64
V, E, H, L, A = 32000, 512, 512, 4, 256
NCORES = 8
BC = B // NCORES            # 8 batch rows per core
TOK = T * BC                # 4096 tokens per core, (t, b) order
G3 = 3 * H                  # 1536 gate rows
MCH = G3 // 128             # 12 gate chunks
KCH = E // 128              # 4 contraction chunks (E == H)
ACH = A // 128              # 2 attention chunks
HT = H // 128               # 4 hidden chunks
UNROLL = 16
SLAB = 512                  # tokens per prologue gemm slab

# attention pair-block offsets for i=0..2 (i=3 is identity); block i holds
# columns (b, k) for k in [i, 4), b-major; block size (4-i)*BC
_OFF = [0, 4 * BC, 7 * BC]
_ETOT = 9 * BC              # 72


def _bcast(ap, dim, count):
    """Insert a [step=0, count] free dim at position `dim` (0=partition)."""
    new = list(ap.ap)
    new.insert(dim, [0, count])
    return bass.AP(tensor=ap.tensor, offset=ap.offset, ap=new)


def _build_kernel():
    nc = bacc.Bacc("TRN2", target_bir_lowering=False, debug=False)

    tokens_d = nc.dram_tensor("tokens32", [TOK // 128, 128], I32, kind="ExternalInput").ap()
    emb_d = nc.dram_tensor("embbf", [V, E], BF16, kind="ExternalInput").ap()
    wih0_d = nc.dram_tensor("wih0", [128, KCH, MCH, 128], BF16, kind="ExternalInput").ap()
    wih_d = nc.dram_tensor("wih", [L - 1, 128, KCH, MCH, 128], BF16, kind="ExternalInput").ap()
    whh_d = nc.dram_tensor("whh", [L, 128, KCH, MCH, 128], BF16, kind="ExternalInput").ap()
    wa_d = nc.dram_tensor("wa", [3, 128, KCH, ACH, 128], BF16, kind="ExternalInput").ap()
    va_d = nc.dram_tensor("vastk", [128, ACH, 3], BF16, kind="ExternalInput").ap()
    balhs_d = nc.dram_tensor("balhs", [6, 128], BF16, kind="ExternalInput").ap()
    bamask_d = nc.dram_tensor("bamask", [6, 2 * _ETOT], BF16, kind="ExternalInput").ap()
    bankb_d = nc.dram_tensor("bankbias", [16, L, 128], BF16, kind="ExternalInput").ap()
    mask16_d = nc.dram_tensor("mask16", [16, 16 * BC], BF16, kind="ExternalInput").ap()
    out_d = nc.dram_tensor("out", [T * BC, H], BF16, kind="ExternalOutput").ap()

    with tile.TileContext(nc) as tc:
        _emit(tc, nc, tokens_d, emb_d, wih0_d, wih_d, whh_d, wa_d, va_d,
              balhs_d, bamask_d, bankb_d, mask16_d, out_d)
    nc.compile()
    return nc


def _emit(tc, nc, tokens_d, emb_d, wih0_d, wih_d, whh_d, wa_d, va_d,
          balhs_d, bamask_d, bankb_d, mask16_d, out_d):
    from contextlib import ExitStack

    ctx = ExitStack()
    with ctx:
        wpool = ctx.enter_context(tc.tile_pool(name="weights", bufs=1))
        state = ctx.enter_context(tc.tile_pool(name="state", bufs=1))
        dram = ctx.enter_context(tc.tile_pool(name="dram", bufs=1, space="DRAM"))

        # ---- resident weights -------------------------------------------
        wih_sb = []
        for l in range(L - 1):
            w = wpool.tile([128, KCH, MCH, 128], BF16, tag=f"wih{l}")
            nc.sync.dma_start(out=w, in_=wih_d[l])
            wih_sb.append(w)
        whh_sb = []
        for l in range(L):
            w = wpool.tile([128, KCH, MCH, 128], BF16, tag=f"whh{l}")
            nc.sync.dma_start(out=w, in_=whh_d[l])
            whh_sb.append(w)
        wa_sb = []
        for i in range(3):
            w = wpool.tile([128, KCH, ACH, 128], BF16, tag=f"wa{i}")
            nc.sync.dma_start(out=w, in_=wa_d[i])
            wa_sb.append(w)
        va_sb = wpool.tile([128, ACH, 3], BF16, tag="va")
        nc.sync.dma_start(out=va_sb, in_=va_d)
        balhs_sb = wpool.tile([6, 128], BF16, tag="balhs")
        nc.sync.dma_start(out=balhs_sb, in_=balhs_d)
        bamask_sb = wpool.tile([6, 2 * _ETOT], BF16, tag="bamask")
        nc.sync.dma_start(out=bamask_sb, in_=bamask_d)
        bankb_sb = wpool.tile([16, L, 128], BF16, tag="bankb")
        nc.sync.dma_start(out=bankb_sb, in_=bankb_d)
        mask16_sb = wpool.tile([16, 16 * BC], BF16, tag="mask16")
        nc.sync.dma_start(out=mask16_sb, in_=mask16_d)

        ident = wpool.tile([128, 128], BF16, tag="ident")
        make_identity(nc, ident)
        ones_sb = wpool.tile([1, 128], BF16, tag="ones")
        nc.vector.memset(ones_sb, 1.0)

        # ---- recurrent state (h_bf holds h* = h/2) ----------------------
        h_bf = state.tile([128, HT, BC, L], BF16, tag="h_bf")
        new_bf = state.tile([128, HT, BC, L], BF16, tag="new_bf")
        nc.vector.memset(h_bf, 0.0)
        nc.vector.memset(new_bf, 0.0)

        # gi0[p, m, tok] bf16: precomputed x @ W_ih[0].T (no bias),
        # SBUF-resident (96KB/partition) so the recurrence needs no DMA
        gi0_sb = state.tile([128, MCH, TOK], BF16, tag="gi0")

        # ---- prologue: embedding gather + layer-0 input GEMM ------------
        with tc.tile_pool(name="prolw", bufs=2) as prolw, \
             tc.tile_pool(name="prol", bufs=2) as prol, \
             tc.tile_pool(name="prol_ps", bufs=2, space="PSUM") as prol_ps, \
             tc.tile_pool(name="gemm_ps", bufs=2, space="PSUM") as gemm_ps, \
             tc.tile_pool(name="x0t", bufs=1) as x0tp:
            for slab in range(TOK // SLAB):
                x0t = x0tp.tile([128, KCH, SLAB], BF16, tag="x0t")
                for g in range(SLAB // 128):
                    gt = slab * (SLAB // 128) + g
                    tok_sb = prol.tile([128, 1], I32, tag="tok")
                    nc.sync.dma_start(out=tok_sb, in_=tokens_d[gt, :, None])
                    x0 = prol.tile([128, E], BF16, tag="x0")
                    nc.gpsimd.indirect_dma_start(
                        out=x0, out_offset=None, in_=emb_d,
                        in_offset=bass.IndirectOffsetOnAxis(ap=tok_sb[:, 0:1], axis=0),
                    )
                    for k in range(KCH):
                        pst = prol_ps.tile([128, 128], BF16, space="PSUM", tag="pst")
                        nc.tensor.transpose(out=pst, in_=x0[:, ts(k, 128)], identity=ident)
                        nc.vector.tensor_copy(out=x0t[:, k, ts(g, 128)], in_=pst)
                for m in range(MCH):
                    w0 = prolw.tile([128, KCH, 128], BF16, tag="w0")
                    nc.sync.dma_start(out=w0, in_=wih0_d[:, :, m, :])
                    ps = gemm_ps.tile([128, SLAB], F32, space="PSUM", tag="g0ps")
                    for k in range(KCH):
                        nc.tensor.matmul(
                            out=ps, lhsT=w0[:, k, :], rhs=x0t[:, k, :],
                            start=(k == 0), stop=(k == KCH - 1),
                        )
                    nc.scalar.activation(out=gi0_sb[:, m, ts(slab, SLAB)],
                                         in_=ps, func=AF.Copy)

        # ---- main recurrence --------------------------------------------
        loop_pools = ExitStack()
        with loop_pools:
            pgp = loop_pools.enter_context(tc.tile_pool(name="pg", bufs=1, space="PSUM"))
            ep = loop_pools.enter_context(tc.tile_pool(name="elem", bufs=3))
            ap_ = loop_pools.enter_context(tc.tile_pool(name="attn", bufs=2))

            # PSUM: 4 gate banks (each holds rz[(j+1)%4] | n[j]) + 1 attn bank
            pgb = [pgp.tile([128, 128], F32, space="PSUM", tag=f"pgb{j}",
                            name=f"pgb{j}")
                   for j in range(L)]
            attn_bank = pgp.tile([128, 512], F32, space="PSUM", tag="attnb")

            def pg_rz(l):
                return pgb[(l + 3) % 4][:, 0:64].rearrange("p (m b) -> p m b", b=BC)

            def pg_n(l):
                return pgb[l][:, 64:128].rearrange("p (m b) -> p m b", b=BC)

            u_ps = attn_bank[:, 0:2 * _ETOT].rearrange("p (a c) -> p a c", c=_ETOT)
            e_ps = attn_bank[0:1, 2 * _ETOT:3 * _ETOT]
            abc_ps = attn_bank[:, 3 * _ETOT:4 * _ETOT]

            st = {"pgb": pgb, "u_ps": u_ps, "e_ps": e_ps, "abc_ps": abc_ps,
                  "pg_rz": pg_rz, "pg_n": pg_n, "h_bf": h_bf, "new_bf": new_bf,
                  "wih_sb": wih_sb, "whh_sb": whh_sb, "wa_sb": wa_sb,
                  "va_sb": va_sb, "balhs_sb": balhs_sb, "bamask_sb": bamask_sb,
                  "bankb_sb": bankb_sb, "mask16_sb": mask16_sb,
                  "ident": ident, "ones_sb": ones_sb, "gi0_sb": gi0_sb,
                  "out_d": out_d, "ep": ep, "ap": ap_}

            with tc.For_i(0, TOK, BC * UNROLL,
                          hint_engines=(mybir.EngineType.PE,
                                        mybir.EngineType.DVE,
                                        mybir.EngineType.Activation)) as iv:
                for u in range(UNROLL):
                    _step(tc, nc, iv + u * BC, st)


def _bank_open(nc, st, j):
    """One start=True matmul seeding bank j with its gate biases.

    Each PSUM bank holds [rz of layer (j+1)%4 | n of layer j]; a start=True
    matmul clears has_written for the WHOLE bank, so exactly one opener per
    bank per step writes all 128 columns (both bias regions) and every other
    matmul into the bank accumulates with start=False."""
    nc.tensor.matmul(
        out=st["pgb"][j], lhsT=st["bankb_sb"][:, j, :], rhs=st["mask16_sb"],
        start=True, stop=False, skip_group_check=True)


def _gh_mm(nc, st, l, mts):
    """gh accumulation for layer l over m-tiles mts (rhs = h* state)."""
    rhs = st["h_bf"][:, :, :, l]
    for mt in mts:
        out = st["pg_rz"](l)[:, mt, :] if mt < 8 else st["pg_n"](l)[:, mt - 8, :]
        for k in range(KCH):
            nc.tensor.matmul(
                out=out, lhsT=st["whh_sb"][l][:, k, mt, :], rhs=rhs[:, k, :],
                start=False, stop=(k == KCH - 1), skip_group_check=True)


def _gi_mm(nc, st, l):
    """gi accumulation for layer l>=1 (rhs = new[l-1]); rz tiles first."""
    rhs = st["new_bf"][:, :, :, l - 1]
    for mt in range(MCH):
        out = st["pg_rz"](l)[:, mt, :] if mt < 8 else st["pg_n"](l)[:, mt - 4, :]
        for k in range(KCH):
            nc.tensor.matmul(
                out=out, lhsT=st["wih_sb"][l - 1][:, k, mt, :], rhs=rhs[:, k, :],
                start=False, stop=(k == KCH - 1), skip_group_check=True)


def _gi0_inject(nc, st, tb0):
    """Accumulate the precomputed layer-0 input gates into PSUM via I.T@x."""
    gi = st["gi0_sb"][:, :, ds(tb0, BC)]
    nc.tensor.matmul(
        out=st["pg_rz"](0).rearrange("p m b -> p (m b)"), lhsT=st["ident"],
        rhs=gi[:, 0:8, :], start=False, stop=True, skip_group_check=True)
    nc.tensor.matmul(
        out=st["pg_n"](0)[:, 4:8, :].rearrange("p m b -> p (m b)"),
        lhsT=st["ident"],
        rhs=gi[:, 8:12, :], start=False, stop=True, skip_group_check=True)


def _elem(nc, st, l, post_npre=None):
    """GRU cell elementwise for layer l. t := tanh(x/2); sigma(x)=(1+t)/2.
    pg_n slots 0..3 hold H2 = (gh_n + b_hn)/2; slots 4..7 hold gi_n + b_in.
    h_bf holds h* = h/2, so z*h = (t_z+1)*h*.
    `post_npre` emits PE work that writes this layer's n-bank (the next
    layer's rz region) right after the last read of that bank, so the bank
    tracker's serialization edge costs nothing."""
    ep = st["ep"]
    trz = ep.tile([128, 8, BC], F32, tag="trz")
    nc.scalar.activation(out=trz, in_=st["pg_rz"](l), func=AF.Tanh, scale=0.5)
    rh = ep.tile([128, HT, BC], F32, tag="rh")
    nc.vector.scalar_tensor_tensor(
        out=rh, in0=trz[:, 0:4, :], scalar=1.0, in1=st["pg_n"](l)[:, 0:4, :],
        op0=ALU.add, op1=ALU.mult)
    npre = ep.tile([128, HT, BC], F32, tag="npre")
    nc.vector.tensor_tensor(out=npre, in0=rh, in1=st["pg_n"](l)[:, 4:8, :],
                            op=ALU.add)
    if post_npre is not None:
        post_npre()
    n = ep.tile([128, HT, BC], F32, tag="n")
    nc.scalar.activation(out=n, in_=npre, func=AF.Tanh)
    zht = ep.tile([128, HT, BC], F32, tag="zht")
    nc.vector.scalar_tensor_tensor(
        out=zht, in0=trz[:, 4:8, :], scalar=1.0, in1=st["h_bf"][:, :, :, l],
        op0=ALU.add, op1=ALU.mult)
    am = ep.tile([128, HT, BC], F32, tag="am")
    nc.vector.scalar_tensor_tensor(
        out=am, in0=trz[:, 4:8, :], scalar=1.0, in1=n,
        op0=ALU.subtract, op1=ALU.mult)
    nc.vector.scalar_tensor_tensor(
        out=st["new_bf"][:, :, :, l], in0=am, scalar=-0.5, in1=zht,
        op0=ALU.mult, op1=ALU.add)


def _attn_pe_u(nc, st):
    """Attention u = Wa.T @ new (+ba) for the *previous* step's new."""
    nc.tensor.matmul(
        out=st["u_ps"].rearrange("p a c -> p (a c)"), lhsT=st["balhs_sb"],
        rhs=st["bamask_sb"], start=True, stop=False, skip_group_check=True)
    for i in range(3):
        sz = (L - i) * BC
        for a2 in range(ACH):
            for k in range(KCH):
                nc.tensor.matmul(
                    out=st["u_ps"][:, a2, _OFF[i]:_OFF[i] + sz],
                    lhsT=st["wa_sb"][i][:, k, a2, :],
                    rhs=st["new_bf"][:, k, :, i:L],
                    start=False, stop=(k == KCH - 1), skip_group_check=True)


def _attn_pe_edot(nc, st, ut):
    for i in range(3):
        sz = (L - i) * BC
        for a2 in range(ACH):
            nc.tensor.matmul(out=st["e_ps"][0:1, _OFF[i]:_OFF[i] + sz],
                             lhsT=st["va_sb"][:, a2, i:i + 1],
                             rhs=ut[:, a2, _OFF[i]:_OFF[i] + sz],
                             start=(a2 == 0), stop=(a2 == ACH - 1),
                             skip_group_check=True)


def _step(tc, nc, tb0, st):
    """Emit one recurrence step for token-block offset tb0.  The attention
    + combine emitted at the top operates on the PREVIOUS step's `new`
    (index-free), interleaved with this step's early h-independent PE work."""
    ap_ = st["ap"]
    ep = st["ep"]
    h_bf, new_bf = st["h_bf"], st["new_bf"]

    # ---- bank openers (biases), then attention(prev) + early gh3 -----
    for j in range(L):
        _bank_open(nc, st, j)
    _attn_pe_u(nc, st)
    ut = ap_.tile([128, ACH, _ETOT], BF16, tag="ut")
    nc.scalar.activation(out=ut, in_=st["u_ps"], func=AF.Tanh)
    _gh_mm(nc, st, 3, range(0, 4))
    _attn_pe_edot(nc, st, ut)
    _gh_mm(nc, st, 3, range(4, 8))
    ee = ap_.tile([1, _ETOT], F32, tag="ee")
    nc.scalar.activation(out=ee, in_=st["e_ps"], func=AF.Exp)
    s_all = ap_.tile([1, 4, BC], F32, tag="sall")
    for i in range(3):
        kk = L - i
        nc.vector.tensor_reduce(
            out=s_all[0:1, i, :],
            in_=ee[0:1, _OFF[i]:_OFF[i] + kk * BC].rearrange(
                "p (b k) -> p b k", k=kk),
            axis=mybir.AxisListType.X, op=ALU.add)
    rs = ap_.tile([1, 4, BC], F32, tag="rs")
    nc.vector.reciprocal(out=rs[0:1, 0:3, :], in_=s_all[0:1, 0:3, :])
    # a' = a/2 so the combine directly produces h* = h/2
    a_bf = ap_.tile([1, _ETOT], BF16, tag="abf")
    for i in range(3):
        kk = L - i
        nc.vector.scalar_tensor_tensor(
            out=a_bf[0:1, _OFF[i]:_OFF[i] + kk * BC].rearrange(
                "p (b k) -> p b k", k=kk),
            in0=ee[0:1, _OFF[i]:_OFF[i] + kk * BC].rearrange(
                "p (b k) -> p b k", k=kk),
            scalar=0.5,
            in1=_bcast(rs[0:1, i, :], 2, kk),
            op0=ALU.mult, op1=ALU.mult)
    nc.tensor.matmul(out=st["abc_ps"], lhsT=st["ones_sb"], rhs=a_bf,
                     start=True, stop=True, skip_group_check=True)
    _gh_mm(nc, st, 3, range(8, 12))
    # combine: h*[i] = sum_k a'[i,k] new[k]   (i=0 first -> unblocks gh0)
    for i in range(3):
        kk = L - i
        prod = ep.tile([128, HT, BC, L], F32, tag="prod")
        av = st["abc_ps"][:, _OFF[i]:_OFF[i] + kk * BC].rearrange(
            "p (b k) -> p b k", k=kk)
        nc.vector.tensor_tensor(
            out=prod[:, :, :, 0:kk], in0=new_bf[:, :, :, i:L],
            in1=_bcast(av, 1, HT), op=ALU.mult)
        with nc.allow_low_precision(reason="<=4-term fp32-internal reduce"):
            nc.vector.tensor_reduce(out=h_bf[:, :, :, i],
                                    in_=prod[:, :, :, 0:kk],
                                    axis=mybir.AxisListType.X, op=ALU.add)

    # ---- this step's gate GEMMs --------------------------------------
    _gi0_inject(nc, st, tb0)
    _gh_mm(nc, st, 0, range(MCH))
    _gh_mm(nc, st, 1, range(MCH))
    _gh_mm(nc, st, 2, range(MCH))

    _elem(nc, st, 0)
    _gi_mm(nc, st, 1)
    _elem(nc, st, 1)
    _gi_mm(nc, st, 2)
    _elem(nc, st, 2)
    _gi_mm(nc, st, 3)
    _elem(nc, st, 3)

    # h*[3] = new[3]/2 (i=3 softmax is the identity)
    nc.vector.tensor_scalar_mul(out=h_bf[:, :, :, 3],
                                in0=new_bf[:, :, :, 3], scalar1=0.5)

    # output row block: out[(t,b), :] = new[3]
    out_stage = ap_.tile([128, BC, HT], BF16, tag="ostage")
    nc.vector.tensor_copy(out=out_stage,
                          in_=new_bf[:, :, :, 3].rearrange("p ht b -> p b ht"))
    nc.sync.dma_start(
        out=st["out_d"][ds(tb0, BC), :].rearrange("b (ht p) -> p b ht", p=128),
        in_=out_stage,
    )


_NC_CACHE = {}


def _get_nc():
    if "nc" not in _NC_CACHE:
        _NC_CACHE["nc"] = _build_kernel()
    return _NC_CACHE["nc"]


def _prep_inputs(tokens, emb, W_ih, W_hh, b_ih, b_hh, Wa, ba, va):
    """Host-side input marshalling (weight layout/dtype only, no compute)."""
    bf = ml_dtypes.bfloat16
    emb_bf = np.ascontiguousarray(np.asarray(emb, np.float32).astype(bf))

    def lhsT_layout(wT):  # [K, M] -> [128, KCH, MCH, 128]
        K, M = wT.shape
        return np.ascontiguousarray(
            wT.reshape(K // 128, 128, M // 128, 128).transpose(1, 0, 2, 3).astype(bf))

    wih_t = [lhsT_layout(np.asarray(W_ih[l], np.float32).T) for l in range(L)]
    # h is stored as h* = h/2: r/z rows of W_hh need x2, n rows x1
    whh_scaled = []
    for l in range(L):
        w = np.asarray(W_hh[l], np.float32).copy()
        w[:2 * H, :] *= 2.0
        whh_scaled.append(lhsT_layout(w.T))
    wa_t = [lhsT_layout(np.asarray(Wa[i], np.float32)) for i in range(3)]
    va_s = np.ascontiguousarray(
        np.asarray(va[:3], np.float32).T.reshape(ACH, 128, 3)
        .transpose(1, 0, 2).astype(bf))

    bsum = np.asarray(b_ih, np.float32) + np.asarray(b_hh, np.float32)
    brz = bsum[:, :1024].reshape(L, 8, 128)            # [l, m, p]
    bhn = np.asarray(b_hh, np.float32)[:, 1024:].reshape(L, 4, 128) * 0.5
    bin_ = np.asarray(b_ih, np.float32)[:, 1024:].reshape(L, 4, 128)
    bn = np.concatenate([bhn, bin_], axis=1)           # [l, 8, 128]
    # bank j holds [rz of layer (j+1)%4 | n of layer j]
    bankbias = np.zeros((16, L, 128), np.float32)      # [row, j, p]
    for j in range(L):
        bankbias[0:8, j] = brz[(j + 1) % L]
        bankbias[8:16, j] = bn[j]
    bankbias = np.ascontiguousarray(bankbias.astype(bf))

    mask16 = np.zeros((16, 16 * BC), np.float32)
    for m in range(16):
        mask16[m, m * BC:(m + 1) * BC] = 1.0
    mask16 = mask16.astype(bf)

    # ba bias for u_ps: row j = a2*3+i -> ba[i, a2*128+p]
    ba_f = np.asarray(ba, np.float32)
    balhs = np.zeros((6, 128), np.float32)
    bamask = np.zeros((6, 2 * _ETOT), np.float32)
    for a2 in range(ACH):
        for i in range(3):
            j = a2 * 3 + i
            balhs[j] = ba_f[i, a2 * 128:(a2 + 1) * 128]
            sz = (L - i) * BC
            bamask[j, a2 * _ETOT + _OFF[i]:a2 * _ETOT + _OFF[i] + sz] = 1.0
    return (emb_bf, wih_t, whh_scaled, wa_t, va_s, bankbias,
            mask16, balhs.astype(bf), bamask.astype(bf))


def kernel(tokens, emb, W_ih, W_hh, b_ih, b_hh, Wa, ba, va):
    nc = _get_nc()
    (emb_bf, wih_t, whh_t, wa_t, va_s, bankbias, mask16, balhs,
     bamask) = _prep_inputs(tokens, emb, W_ih, W_hh, b_ih, b_hh, Wa, ba, va)

    tok = np.asarray(tokens).astype(np.int32)  # [T, B]
    wih_arr = np.stack(wih_t[1:])
    whh_arr = np.stack(whh_t)
    wa_arr = np.stack(wa_t)

    in_maps = []
    for c in range(NCORES):
        tok_c = np.ascontiguousarray(
            tok[:, c * BC:(c + 1) * BC]).reshape(TOK // 128, 128)
        in_maps.append({
            "tokens32": tok_c,
            "embbf": emb_bf,
            "wih0": wih_t[0],
            "wih": wih_arr,
            "whh": whh_arr,
            "wa": wa_arr,
            "vastk": va_s,
            "balhs": balhs,
            "bamask": bamask,
            "bankbias": bankbias,
            "mask16": mask16,
        })

    trace = bool(int(os.environ.get("KERNEL_TRACE", "0")))
    res = run_bass_kernel_spmd(nc, in_maps, core_ids=list(range(NCORES)),
                               trace=trace)
    if trace:
        _NC_CACHE["last_exec_time_ns"] = res.exec_time_ns
        _NC_CACHE["last_results"] = res

    outs = []
    for c in range(NCORES):
        o = res.results[c]["out"].astype(np.float32).reshape(T, BC, H)
        outs.append(o)
    return np.concatenate(outs, axis=1)
